# revision 28
# baseline (speedup 1.0000x reference)
"""KGAT 2-layer GNN message passing on 8 trn2 NeuronCores (Bass/Tile).

Sharding: destination-row partition. Each core owns 20000 destination rows and
the edges pointing into them. Host pre-buckets edges by (dest 128-row block,
source range of 32768 rows) — bucket tile counts are maxed across cores so one
SPMD program serves all 8 — and pre-expands the val-scaled one-hot scatter
matrices P (blocked-ELL expansion of A) in bf16.

On device, per group of NB dest blocks: R dma_gather instructions (one per
source range; int16 indices) fetch all needed source rows into SBUF slots
(k%128 -> partition, k//128 -> slot column), and one direct DMA streams P.
Per block, T bf16 matmuls accumulate side^T in PSUM; the bi-interaction MLP is
one block-diagonal matmul + Relu-composed leaky; h1+h2 and the transpose fuse
into a single matmul against [I;I]; L2-normalize uses Sqrt(ss+1e-24) +
reciprocal. Outputs are staged per group and stored column-blocked
([128, NBLK*DO]); the host de-permutes. The inter-layer redistribution of ego
happens on the host between the two layer NEFFs.
"""
import hashlib
import os

import numpy as np

import concourse.bass as bass
import concourse.mybir as mybir
import concourse.tile as tile
from concourse import bacc, library_config
from concourse.bass_utils import run_bass_kernel_spmd

N = 160000
E = 2560000
NC = 8
SHARD = N // NC          # 20000
BW = 128                 # dest block width
NBLK = -(-SHARD // BW)   # 157 (last block has 32 valid rows)
NB = 6                   # dest blocks per DMA group
RANGE = 32768            # source rows per dma_gather range (int16 index limit)
XPAD = 128               # padded x row length in bf16 (256B gather elements)

F32 = mybir.dt.float32
HALF = mybir.dt.float16
I16 = mybir.dt.int16
HDT = np.float16

_cache = {}
LAST_EXEC_NS = None
_TRACE = bool(os.environ.get("KGAT_TRACE"))


def _prep_edges(edge_row, edge_col, edge_val, n_nodes=N, nc_=NC, shard=SHARD,
                nblk=NBLK, rng_sz=RANGE):
    """Bucket edges by (core, dest block, source range); expand scatter mats.

    Returns (gidx, pall, struct):
      gidx [nc_, 128, 8*ST] int16 — dma_gather index streams (16-wrapped,
           replicated to 128 partitions); instruction (g, r) uses columns
           [8*base, 8*(base+J)).
      pall [nc_, 128, ST*128] bf16 — val-scaled one-hot per slot.
      struct — static schedule shared by all cores (hashable).
    """
    nrg = -(-n_nodes // rng_sz)
    core = edge_row // shard
    rloc = edge_row - core * shard
    blk = rloc // BW
    rl = (rloc - blk * BW).astype(np.int64)
    rng_ = edge_col // rng_sz
    col_local = (edge_col - rng_ * rng_sz).astype(np.int16)
    key = (core * nblk + blk) * nrg + rng_

    # secondary sort by source: within a bucket, slots walk ascending HBM
    # addresses, which improves gather descriptor locality
    order = np.lexsort((edge_col, key))
    key_s = key[order]
    cl_s = col_local[order]
    rl_s = rl[order]
    val_s = edge_val[order].astype(np.float32)

    counts = np.bincount(key_s, minlength=nc_ * nblk * nrg)
    counts = counts.reshape(nc_, nblk, nrg)
    Tbr = (-(-counts.max(axis=0) // 128)).astype(np.int64)   # [nblk, nrg]
    # ensure every block has at least one tile (degenerate safety)
    empty = Tbr.sum(axis=1) == 0
    Tbr[empty, 0] = 1

    groups = [(b0, min(NB, nblk - b0)) for b0 in range(0, nblk, NB)]
    # tile-column assignment: group -> range -> block
    tcol_base = np.zeros((nblk, nrg), np.int64)
    ginfo = []
    pos = 0
    for b0, nb in groups:
        g_start = pos
        rinfo = []
        for r in range(nrg):
            r_start = pos
            for b in range(b0, b0 + nb):
                tcol_base[b, r] = pos
                pos += Tbr[b, r]
            rinfo.append((r_start, pos - r_start))  # (base_gr, J_gr)
        ginfo.append((b0, nb, g_start, pos - g_start, rinfo))
    ST = int(pos)

    starts = np.zeros(nc_ * nblk * nrg, np.int64)
    np.cumsum(counts.ravel()[:-1], out=starts[1:])
    rank = np.arange(len(key_s), dtype=np.int64) - starts[key_s]
    lane = rank % 128
    t = rank // 128
    c_s = key_s // (nblk * nrg)
    b_s = (key_s // nrg) % nblk
    r_s = key_s % nrg
    tcol = tcol_base[b_s, r_s] + t

    pall = np.zeros((nc_, 128, ST * 128), HDT)
    pall[c_s, lane, tcol * 128 + rl_s] = val_s.astype(HDT)

    # dma_gather index streams: within instruction (g, r), flat position
    # f = (tcol - base_gr)*128 + lane ; idx16[f%16, 8*base_gr + f//16]
    base_gr_of = np.zeros((nblk, nrg), np.int64)
    for b0, nb, g_start, g_len, rinfo in ginfo:
        for r, (r_start, J) in enumerate(rinfo):
            base_gr_of[b0:b0 + nb, r] = r_start
    f = (tcol - base_gr_of[b_s, r_s]) * 128 + lane
    gidx16 = np.zeros((nc_, 16, 8 * ST), np.int16)
    gidx16[c_s, f % 16, 8 * base_gr_of[b_s, r_s] + f // 16] = cl_s
    gidx = np.ascontiguousarray(np.tile(gidx16, (1, 8, 1)))

    # per (group, block): local tile columns, for the matmul schedule
    sched = []
    for b0, nb, g_start, g_len, rinfo in ginfo:
        blocks = []
        for b in range(b0, b0 + nb):
            cols = []
            for r in range(nrg):
                for t_ in range(Tbr[b, r]):
                    cols.append(int(tcol_base[b, r] - g_start + t_))
            blocks.append(tuple(cols))
        sched.append((b0, nb, int(g_start), int(g_len),
                      tuple((int(rs - g_start), int(J)) for rs, J in rinfo),
                      tuple(blocks)))
    struct = (nrg, ST, tuple(sched))
    return gidx, pall, struct


def _build_layer(D, DO, struct, emit_ego, n_nodes=N, nblk=NBLK, ndev=NC,
                 rng_sz=RANGE, has_bias=False):
    """One layer's Bacc program: x[n_nodes, XPAD] -> norm/ego [128, nblk*DO]."""
    nrg, ST, sched = struct
    JGMAX = max(g_len for _, _, _, g_len, _, _ in sched)
    NBMAX = max(nb for _, nb, _, _, _, _ in sched)

    nc = bacc.Bacc("TRN2", target_bir_lowering=False, debug=False,
                   num_devices=ndev, num_swdge_queues=4)
    xw = nc.dram_tensor("xw", [n_nodes, XPAD], HALF, kind="ExternalInput")
    xt = nc.dram_tensor("xt", [D, nblk * 128], HALF, kind="ExternalInput")
    gidx = nc.dram_tensor("gidx", [128, 8 * ST], I16, kind="ExternalInput")
    pall = nc.dram_tensor("pall", [128, ST * 128], HALF, kind="ExternalInput")
    wcat = nc.dram_tensor("wcat", [2 * D, 2 * DO], F32, kind="ExternalInput")
    bcat = nc.dram_tensor("bcat", [2 * DO, 1], F32, kind="ExternalInput")
    ident2 = nc.dram_tensor("ident2", [2 * DO, DO], F32, kind="ExternalInput")
    if has_bias:
        braw = nc.dram_tensor("braw", [2 * DO, 1], F32, kind="ExternalInput")
    norm_o = nc.dram_tensor("norm_o", [128, nblk * DO], F32,
                            kind="ExternalOutput")
    if emit_ego:
        ego_o = nc.dram_tensor("ego_o", [128, nblk * DO], HALF,
                               kind="ExternalOutput")

    AF = mybir.ActivationFunctionType
    OP = mybir.AluOpType
    _qrr = [0]
    with tile.TileContext(nc) as tc:
        with tc.tile_pool(name="const", bufs=1) as cp, \
             tc.tile_pool(name="meta", bufs=2) as mp, \
             tc.tile_pool(name="gath", bufs=2) as gp, \
             tc.tile_pool(name="pmat", bufs=2) as pp, \
             tc.tile_pool(name="work", bufs=4) as wp, \
             tc.tile_pool(name="stage", bufs=2) as op_, \
             tc.tile_pool(name="ps_s", bufs=2, space="PSUM") as ps_s, \
             tc.tile_pool(name="ps_h", bufs=2, space="PSUM") as ps_h, \
             tc.tile_pool(name="ps_e", bufs=2, space="PSUM") as ps_e:

            nc.gpsimd.load_library(library_config.mlp)

            wcat_t = cp.tile([2 * D, 2 * DO], F32)
            nc.sync.dma_start(wcat_t[:], wcat[:, :])
            bcat_t = cp.tile([2 * DO, 1], F32)
            nc.sync.dma_start(bcat_t[:], bcat[:, :])
            if has_bias:
                braw_t = cp.tile([2 * DO, 1], F32)
                nc.sync.dma_start(braw_t[:], braw[:, :])
            id2_t = cp.tile([2 * DO, DO], F32)
            nc.sync.dma_start(id2_t[:], ident2[:, :])
            eps_t = cp.tile([128, 1], F32)
            nc.vector.memset(eps_t[:], 1e-24)

            for b0, nb, g_start, g_len, rinfo, blocks in sched:
                xtg = mp.tile([D, NBMAX * 128], HALF, tag="xtg")
                nc.sync.dma_start(xtg[:, :nb * 128],
                                  xt[:, b0 * 128:(b0 + nb) * 128])
                gix = mp.tile([128, 8 * JGMAX], I16, tag="gix")
                nc.sync.dma_start(gix[:, :8 * g_len],
                                  gidx[:, 8 * g_start:8 * (g_start + g_len)])
                pg = pp.tile([128, JGMAX * 128], HALF, tag="pg")
                nc.sync.dma_start(pg[:, :g_len * 128],
                                  pall[:, g_start * 128:(g_start + g_len) * 128])
                xg = gp.tile([128, JGMAX * 128], HALF, tag="xg")
                for r in range(nrg):
                    loc, J = rinfo[r]
                    if J == 0:
                        continue
                    out_ap = xg[:, loc * 128:(loc + J) * 128].rearrange(
                        "p (j e) -> p j e", e=128)
                    hi = min((r + 1) * rng_sz, n_nodes)
                    nc.gpsimd.dma_gather(
                        out_ap, xw[r * rng_sz:hi, :],
                        gix[:, 8 * loc:8 * (loc + J)],
                        128 * J, 128 * J, XPAD, single_packet=False,
                        queue_num=_qrr[0] % 4)
                    _qrr[0] += 1

                st_n = op_.tile([128, NBMAX * DO], F32, tag="stn")
                st_e = op_.tile([128, NBMAX * DO], HALF, tag="ste")

                for k in range(nb):
                    b = b0 + k
                    cols = blocks[k]
                    side = ps_s.tile([D, 128], F32, tag="side")
                    for i, j in enumerate(cols):
                        nc.tensor.matmul(
                            out=side[:],
                            lhsT=xg[:, j * 128:j * 128 + D],
                            rhs=pg[:, j * 128:(j + 1) * 128],
                            start=(i == 0), stop=(i == len(cols) - 1),
                        )
                    ego_in = xtg[:, k * 128:(k + 1) * 128]
                    sp = wp.tile([2 * D, 128], F32, tag="sp")
                    nc.vector.tensor_tensor(out=sp[0:D, :], in0=ego_in,
                                            in1=side[:], op=OP.add)
                    nc.vector.tensor_tensor(out=sp[D:2 * D, :], in0=ego_in,
                                            in1=side[:], op=OP.mult)
                    h = ps_h.tile([2 * DO, 128], F32, tag="h")
                    nc.tensor.matmul(out=h[:], lhsT=wcat_t[:], rhs=sp[:],
                                     start=True, stop=True)
                    r5 = wp.tile([2 * DO, 128], F32, tag="r5")
                    nc.scalar.activation(out=r5[:], in_=h[:], func=AF.Relu,
                                         bias=bcat_t[:], scale=0.99)
                    hs = wp.tile([2 * DO, 128], F32, tag="hs")
                    if has_bias:
                        u = wp.tile([2 * DO, 128], F32, tag="u")
                        nc.vector.tensor_scalar(
                            out=u[:], in0=h[:], scalar1=braw_t[:],
                            scalar2=0.01, op0=OP.add, op1=OP.mult)
                        nc.vector.tensor_tensor(out=hs[:], in0=u[:],
                                                in1=r5[:], op=OP.add)
                    else:
                        nc.vector.scalar_tensor_tensor(
                            out=hs[:], in0=h[:], scalar=0.01, in1=r5[:],
                            op0=OP.mult, op1=OP.add)
                    egops = ps_e.tile([128, DO], F32, tag="egops")
                    nc.tensor.matmul(out=egops[:], lhsT=hs[:], rhs=id2_t[:],
                                     start=True, stop=True)
                    er = st_e[:, k * DO:(k + 1) * DO]
                    nc.vector.tensor_copy(er, egops[:])
                    sq = wp.tile([128, DO], HALF, tag="sq")
                    ss = wp.tile([128, 1], F32, tag="ss")
                    nc.vector.scalar_tensor_tensor(
                        out=sq[:], in0=er, scalar=1.0, in1=er,
                        op0=OP.mult, op1=OP.mult, accum_out=ss[:])
                    nrm = wp.tile([128, 1], F32, tag="nrm")
                    nc.scalar.activation(out=nrm[:], in_=ss[:], func=AF.Sqrt,
                                         bias=eps_t[:])
                    rinv = wp.tile([128, 1], F32, tag="rinv")
                    nc.vector.reciprocal(rinv[:], nrm[:])
                    nc.scalar.activation(out=st_n[:, k * DO:(k + 1) * DO],
                                         in_=er, func=AF.Copy, bias=0.0,
                                         scale=rinv[:])

                nc.sync.dma_start(norm_o[:, b0 * DO:(b0 + nb) * DO],
                                  st_n[:, :nb * DO])
                if emit_ego:
                    nc.sync.dma_start(ego_o[:, b0 * DO:(b0 + nb) * DO],
                                      st_e[:, :nb * DO])

    nc.compile()
    return nc


def _unpack(dev_out, DO, nblk=NBLK, shard=SHARD):
    """[128, nblk*DO] column-blocked device output -> [shard, DO] row-major."""
    return np.ascontiguousarray(
        dev_out.reshape(128, nblk, DO).transpose(1, 0, 2).reshape(-1, DO)[:shard])


def _wcat(W1, W2):
    D, DO = W1.shape
    w = np.zeros((2 * D, 2 * DO), np.float32)
    w[:D, :DO] = np.asarray(W1, np.float32)
    w[D:, DO:] = np.asarray(W2, np.float32)
    return w


def _ident2(DO):
    return np.concatenate([np.eye(DO), np.eye(DO)]).astype(np.float32)


def _bcat(b1, b2):
    """(0.99*b for the ACT relu, raw b, has_bias)."""
    raw = np.ascontiguousarray(
        np.concatenate([np.asarray(b1, np.float32),
                        np.asarray(b2, np.float32)]).reshape(-1, 1))
    return np.ascontiguousarray(0.99 * raw), raw, bool(np.any(raw != 0.0))


def _xpad(x_bf):
    """Pad rows to XPAD bf16 columns (256B dma_gather elements)."""
    out = np.zeros((x_bf.shape[0], XPAD), HDT)
    out[:, :x_bf.shape[1]] = x_bf
    return out


def _xt_pad(x_bf, nblk=NBLK, shard=SHARD, nc_=NC):
    """Per-core padded feature-major ego slices [nc_, D, nblk*128]."""
    D = x_bf.shape[1]
    out = np.zeros((nc_, D, nblk * 128), HDT)
    for c in range(nc_):
        out[c][:, :shard] = x_bf[c * shard:(c + 1) * shard].T
    return out


def kernel(node_embed, edge_row, edge_col, edge_val,
           W1_0, b1_0, W2_0, b2_0, W1_1, b1_1, W2_1, b2_1):
    node_embed = np.asarray(node_embed, np.float32)
    edge_row = np.asarray(edge_row, np.int32)
    edge_col = np.asarray(edge_col, np.int32)
    edge_val = np.asarray(edge_val, np.float32)

    ehash = hashlib.md5(
        edge_row.tobytes() + edge_col.tobytes() + edge_val.tobytes()
    ).hexdigest()
    if ("edges", ehash) not in _cache:
        _cache[("edges", ehash)] = _prep_edges(edge_row, edge_col, edge_val)
    gidx, pall, struct = _cache[("edges", ehash)]

    bcat0, braw0, hb0 = _bcat(b1_0, b2_0)
    bcat1, braw1, hb1 = _bcat(b1_1, b2_1)

    skey = hash(struct)
    if ("L0", skey, hb0) not in _cache:
        _cache[("L0", skey, hb0)] = _build_layer(64, 32, struct,
                                                 emit_ego=True, has_bias=hb0)
    if ("L1", skey, hb1) not in _cache:
        _cache[("L1", skey, hb1)] = _build_layer(32, 16, struct,
                                                 emit_ego=False, has_bias=hb1)
    nc0 = _cache[("L0", skey, hb0)]
    nc1 = _cache[("L1", skey, hb1)]

    x0 = node_embed.astype(HDT)
    xw0 = _xpad(x0)
    xt0 = _xt_pad(x0)
    wcat0 = _wcat(W1_0, W2_0)
    in_maps0 = [{
        "xw": xw0, "xt": xt0[c], "gidx": gidx[c], "pall": pall[c],
        "wcat": wcat0, "bcat": bcat0, "ident2": _ident2(32),
        **({"braw": braw0} if hb0 else {}),
    } for c in range(NC)]
    res0 = run_bass_kernel_spmd(nc0, in_maps0, core_ids=list(range(NC)),
                                trace=_TRACE)

    norm1 = np.concatenate(
        [_unpack(res0.results[c]["norm_o"], 32) for c in range(NC)], axis=0)
    x1 = np.concatenate(
        [_unpack(res0.results[c]["ego_o"], 32) for c in range(NC)], axis=0)

    xw1 = _xpad(x1)
    xt1 = _xt_pad(x1)
    wcat1 = _wcat(W1_1, W2_1)
    in_maps1 = [{
        "xw": xw1, "xt": xt1[c], "gidx": gidx[c], "pall": pall[c],
        "wcat": wcat1, "bcat": bcat1, "ident2": _ident2(16),
        **({"braw": braw1} if hb1 else {}),
    } for c in range(NC)]
    res1 = run_bass_kernel_spmd(nc1, in_maps1, core_ids=list(range(NC)),
                                trace=_TRACE)
    norm2 = np.concatenate(
        [_unpack(res1.results[c]["norm_o"], 16) for c in range(NC)], axis=0)

    global LAST_EXEC_NS
    if res0.exec_time_ns is not None or res1.exec_time_ns is not None:
        LAST_EXEC_NS = (res0.exec_time_ns or 0) + (res1.exec_time_ns or 0)
        globals()["LAST_RES"] = (res0, res1)

    out = np.empty((N, 64 + 32 + 16), np.float32)
    out[:, :64] = node_embed
    out[:, 64:96] = norm1
    out[:, 96:] = norm2
    return out
